# revision 11
# baseline (speedup 1.0000x reference)
"""Trainium2 Bass kernel for a 3-layer GIN encoder (gnn_message_passing).

Reference computation (per layer l):
    agg_i = sum_{j -> i} z_j          (scatter-add over edges)
    h     = z + agg                   (GIN eps=0, folded in as self-edges)
    z     = relu(relu(h @ w1 + b1) @ w2 + b2)

Distribution strategy (8 NeuronCores, SPMD single program):
  * Nodes are block-sharded: core c owns rows [c*NPC, (c+1)*NPC).
  * Edges are partitioned by destination core; scatter-add is local.
  * Each layer's full activation table z (bf16, row-major) lives in DRAM on
    every core (AllGather at layer boundaries = the "halo exchange" in the
    extreme case of a dense random graph).
  * Per destination M-tile of 128 nodes, the ~CPM*128 incoming edges are
    fetched with dma_gather (SWDGE row gather, bf16 rows) and reduced with
    TensorE: for each 128-edge chunk,  aggT += G_chunk.T @ onehot  where
    onehot[p, s] = (dstslot[p] == s) is built on DVE via iota compare.
    This yields h.T (features on partitions), which feeds the MLP directly.
  * MLP runs on groups of 4 M-tiles (512 rows in the free dim) with
    hi/lo-split bf16 matmuls (error ~= fp32) accumulated in fp32 PSUM.
  * Output rows are transposed back via TensorE and DMA'd out; layers 0..L-2
    are AllGathered into the next layer's gather table.
"""

import os
import sys

sys.path.insert(0, "/opt/trn_rl_repo")

import numpy as np
import ml_dtypes

BF16 = ml_dtypes.bfloat16
P = 128
NCORES = 8
PAD_SLOT = 300.0  # bf16-exact, never matches a real slot (0..127)

# number of hi/lo product terms in the MLP matmuls:
# 3 = (w_hi*h_hi + w_hi*h_lo + w_lo*h_hi) ~ fp32 accuracy
# 1 = plain bf16
NSPLIT = 3

# chunks (of 128 edges) per dma_gather sub-call
GSUB = 6

_BUILD_CACHE: dict = {}


# --------------------------------------------------------------------------
# host-side preprocessing
# --------------------------------------------------------------------------

def _config(inputs):
    x = inputs["x"]
    N, DIN = int(x.shape[0]), int(x.shape[1])
    L = 0
    while f"w1_{L}" in inputs:
        L += 1
    DH = int(inputs["w1_0"].shape[1])
    assert N % NCORES == 0
    NPC = N // NCORES
    MT = (NPC + P - 1) // P
    assert DIN % P == 0 and DH % P == 0
    return dict(N=N, DIN=DIN, DH=DH, L=L, NPC=NPC, MT=MT)


def _prep_edges(edge_index, N, NPC, MT):
    """Partition edges (plus one self-edge per node) by destination core,
    sort by destination, pad each (core, mtile) bucket to CPM*128 edges.

    Returns:
      idx_t  [NCORES, 128, MT*CPM*8] int16 -- gather indices, SWDGE wrap
             layout (idx j of a call at [j%16, j//16], replicated to all
             8 groups of 16 partitions), one contiguous block per M-tile.
      slot_t [NCORES, 128, MT*CPM] bf16 -- destination slot (0..127) of edge
             j=c*128+p of M-tile m at [p, m*CPM+c]; PAD_SLOT for padding.
      CPM    int -- chunks (of 128 edges) per M-tile.
    """
    src = np.asarray(edge_index[0], dtype=np.int64)
    dst = np.asarray(edge_index[1], dtype=np.int64)
    self_ix = np.arange(N, dtype=np.int64)
    allsrc = np.concatenate([src, self_ix])
    alldst = np.concatenate([dst, self_ix])

    core = alldst // NPC
    local = alldst - core * NPC
    mt = local // P
    slot = local % P

    counts = np.zeros((NCORES, MT), np.int64)
    np.add.at(counts, (core, mt), 1)
    CPM = int(np.ceil(counts.max() / P))

    order = np.argsort(alldst, kind="stable")
    ssrc, score, smt, sslot = allsrc[order], core[order], mt[order], slot[order]
    gid = score * MT + smt  # sorted ascending
    pos = np.arange(len(gid)) - np.searchsorted(gid, gid, side="left")

    src_arr = np.zeros((NCORES, MT, CPM * P), np.int16)
    slot_arr = np.full((NCORES, MT, CPM * P), PAD_SLOT, np.float32)
    src_arr[score, smt, pos] = ssrc.astype(np.int16)
    slot_arr[score, smt, pos] = sslot

    # SWDGE wrap: per call (M-tile), idx j sits at [j%16, j//16]; replicate
    # the 16-partition block to all 128 partitions (one copy per Q7 core).
    w = src_arr.reshape(NCORES, MT, CPM * 8, 16).transpose(0, 1, 3, 2)
    w = np.broadcast_to(w[:, :, None, :, :], (NCORES, MT, 8, 16, CPM * 8))
    w = w.reshape(NCORES, MT, P, CPM * 8).transpose(0, 2, 1, 3)
    idx_t = np.ascontiguousarray(w.reshape(NCORES, P, MT * CPM * 8))

    s = slot_arr.reshape(NCORES, MT, CPM, P).transpose(0, 3, 1, 2)
    slot_t = np.ascontiguousarray(s.reshape(NCORES, P, MT * CPM)).astype(BF16)
    return idx_t, slot_t, CPM


# --------------------------------------------------------------------------
# bass program
# --------------------------------------------------------------------------

def _build(N, DIN, DH, L, NPC, MT, CPM):
    from concourse import bass, bacc, mybir, tile

    f32 = mybir.dt.float32
    bf = mybir.dt.bfloat16
    i16 = mybir.dt.int16
    EQ = mybir.AluOpType.is_equal
    SUB = mybir.AluOpType.subtract
    RELU = mybir.ActivationFunctionType.Relu

    NKT2 = DH // P  # K/M tiles of the hidden dim (2)
    GROUPS = [(g0, min(4, MT - g0)) for g0 in range(0, MT, 4)]

    nc = bacc.Bacc(num_devices=NCORES)

    xin = nc.dram_tensor("x_bf", [N, DIN], bf, kind="ExternalInput")
    idxin = nc.dram_tensor("srcidx", [P, MT * CPM * 8], i16, kind="ExternalInput")
    slotin = nc.dram_tensor("dstslot", [P, MT * CPM], bf, kind="ExternalInput")
    iotain = nc.dram_tensor("iota", [P, 512], bf, kind="ExternalInput")
    identbin = nc.dram_tensor("identb", [P, P], bf, kind="ExternalInput")
    identfin = nc.dram_tensor("identf", [P, P], f32, kind="ExternalInput")
    win = {}
    for l in range(L):
        din = DIN if l == 0 else DH
        for nm, shp in [
            ("w1h", [din, DH]), ("w1l", [din, DH]),
            ("w2h", [DH, DH]), ("w2l", [DH, DH]),
        ]:
            win[(nm, l)] = nc.dram_tensor(f"{nm}_{l}", shp, bf, kind="ExternalInput")
        for nm in ("b1", "b2"):
            win[(nm, l)] = nc.dram_tensor(f"{nm}_{l}", [DH, 1], f32, kind="ExternalInput")
    zout = nc.dram_tensor("zout", [NPC, DH], f32, kind="ExternalOutput")

    with tile.TileContext(nc) as tc:
        with tc.tile_pool(name="const", bufs=1) as cp, \
             tc.tile_pool(name="gpool", bufs=2) as gp, \
             tc.tile_pool(name="ohpool", bufs=4) as ohp, \
             tc.tile_pool(name="hpool", bufs=1) as hp, \
             tc.tile_pool(name="spool", bufs=2) as sp, \
             tc.tile_pool(name="zpool", bufs=1) as zp, \
             tc.tile_pool(name="zrpool", bufs=3) as zrp, \
             tc.tile_pool(name="aggpsum", bufs=2, space="PSUM") as aggpool, \
             tc.tile_pool(name="mlppsum", bufs=2, space="PSUM") as mlppool, \
             tc.tile_pool(name="tppsum", bufs=2, space="PSUM") as tppool, \
             tc.tile_pool(name="drampool", bufs=1, space="DRAM") as dp:

            # ---------------- resident constants ----------------
            idx_t = cp.tile([P, MT * CPM * 8], i16, name="idx_t")
            nc.sync.dma_start(out=idx_t[:], in_=idxin[:, :])
            slot_t = cp.tile([P, MT * CPM], bf, name="slot_t")
            nc.sync.dma_start(out=slot_t[:], in_=slotin[:, :])
            iota_t = cp.tile([P, 512], bf, name="iota_t")
            nc.sync.dma_start(out=iota_t[:], in_=iotain[:, :])
            identb_t = cp.tile([P, P], bf, name="identb_t")
            nc.sync.dma_start(out=identb_t[:], in_=identbin[:, :])
            identf_t = cp.tile([P, P], f32, name="identf_t")
            nc.sync.dma_start(out=identf_t[:], in_=identfin[:, :])

            wt = {}
            for l in range(L):
                din = DIN if l == 0 else DH
                nkt = din // P
                for nm in ("w1h", "w1l"):
                    t = cp.tile([P, nkt * DH], bf, name=f"{nm}{l}_t")
                    for kt in range(nkt):
                        nc.sync.dma_start(
                            out=t[:, kt * DH:(kt + 1) * DH],
                            in_=win[(nm, l)][kt * P:(kt + 1) * P, :])
                    wt[(nm, l)] = t
                for nm in ("w2h", "w2l"):
                    t = cp.tile([P, NKT2 * DH], bf, name=f"{nm}{l}_t")
                    for kt in range(NKT2):
                        nc.sync.dma_start(
                            out=t[:, kt * DH:(kt + 1) * DH],
                            in_=win[(nm, l)][kt * P:(kt + 1) * P, :])
                    wt[(nm, l)] = t
                for nm in ("b1", "b2"):
                    t = cp.tile([P, NKT2], f32, name=f"{nm}{l}_t")
                    for mo in range(NKT2):
                        nc.sync.dma_start(
                            out=t[:, mo:mo + 1],
                            in_=win[(nm, l)][mo * P:(mo + 1) * P, :])
                    wt[(nm, l)] = t

            # layer-boundary activation tables
            zloc = [dp.tile([NPC, DH], bf, name=f"zloc{l}") for l in range(L - 1)]
            zfull = [dp.tile([N, DH], bf, name=f"zfull{l}", addr_space="Shared")
                     for l in range(L - 1)]

            # ---------------- layers ----------------
            for l in range(L):
                din = DIN if l == 0 else DH
                nkt = din // P
                elem = din
                last = (l == L - 1)
                table = xin[:, :] if l == 0 else zfull[l - 1][:, :]

                hhi = [hp.tile([P, MT * P], bf, name=f"hhi{kt}_{l}", tag=f"hhi{kt}")
                       for kt in range(nkt)]
                hlo = [hp.tile([P, MT * P], bf, name=f"hlo{kt}_{l}", tag=f"hlo{kt}")
                       for kt in range(nkt)]

                # --- aggregation: h.T = (A+I) z, transposed layout
                for m in range(MT):
                    g = gp.tile([P, CPM * elem], bf, name=f"g_{l}_{m}", tag="g")
                    for c0 in range(0, CPM, GSUB):
                        c1 = min(CPM, c0 + GSUB)
                        nc.gpsimd.dma_gather(
                            out_ap=g[:, c0 * elem: c1 * elem]
                                .rearrange("p (c e) -> p c e", e=elem),
                            in_ap=table,
                            idxs_ap=idx_t[:, (m * CPM + c0) * 8:(m * CPM + c1) * 8],
                            num_idxs=(c1 - c0) * P,
                            num_idxs_reg=(c1 - c0) * P,
                            elem_size=elem,
                        )
                    aggps = [aggpool.tile([P, P], f32,
                                          name=f"agg{kt}_{l}_{m}", tag=f"agg{kt}")
                             for kt in range(nkt)]
                    for c0 in range(0, CPM, 4):
                        nchk = min(4, CPM - c0)
                        oh = ohp.tile([P, 512], bf, name=f"oh_{l}_{m}_{c0}", tag="oh")
                        nc.vector.tensor_tensor(
                            out=oh[:, :nchk * P].rearrange("p (c q) -> p c q", q=P),
                            in0=slot_t[:, m * CPM + c0: m * CPM + c0 + nchk, None]
                                .to_broadcast([P, nchk, P]),
                            in1=iota_t[:, :nchk * P].rearrange("p (c q) -> p c q", q=P),
                            op=EQ,
                        )
                        for ci in range(nchk):
                            c = c0 + ci
                            for kt in range(nkt):
                                nc.tensor.matmul(
                                    out=aggps[kt][:, :],
                                    lhsT=g[:, c * elem + kt * P: c * elem + (kt + 1) * P],
                                    rhs=oh[:, ci * P:(ci + 1) * P],
                                    start=(c == 0),
                                    stop=(c == CPM - 1),
                                )
                    for kt in range(nkt):
                        nc.vector.tensor_copy(
                            out=hhi[kt][:, m * P:(m + 1) * P],
                            in_=aggps[kt][:, :])
                        nc.vector.tensor_tensor(
                            out=hlo[kt][:, m * P:(m + 1) * P],
                            in0=aggps[kt][:, :],
                            in1=hhi[kt][:, m * P:(m + 1) * P],
                            op=SUB)

                # --- MLP over groups of 4 M-tiles (512-row free dim)
                zT = [zp.tile([P, MT * P], f32 if last else bf,
                              name=f"zT{mo}_{l}",
                              tag=f"zT{mo}{'f' if last else 'b'}")
                      for mo in range(NKT2)]
                for (g0, gm) in GROUPS:
                    rows = gm * P
                    r0 = g0 * P
                    combos1 = [("w1h", hhi), ("w1h", hlo), ("w1l", hhi)][:NSPLIT]
                    s1h, s1l = [], []
                    for mo in range(NKT2):
                        p1 = mlppool.tile([P, 512], f32,
                                          name=f"p1_{l}_{g0}_{mo}", tag="mlp")
                        tot = len(combos1) * nkt
                        step = 0
                        for (wn, ht) in combos1:
                            for kt in range(nkt):
                                nc.tensor.matmul(
                                    out=p1[:, :rows],
                                    lhsT=wt[(wn, l)][:, kt * DH + mo * P: kt * DH + (mo + 1) * P],
                                    rhs=ht[kt][:, r0:r0 + rows],
                                    start=(step == 0), stop=(step == tot - 1))
                                step += 1
                        s1f = sp.tile([P, 512], f32, name=f"s1f_{l}_{g0}_{mo}", tag="s1f")
                        nc.scalar.activation(
                            out=s1f[:, :rows], in_=p1[:, :rows], func=RELU,
                            bias=wt[("b1", l)][:, mo:mo + 1])
                        sh = sp.tile([P, 512], bf, name=f"s1h_{l}_{g0}_{mo}", tag=f"s1h{mo}")
                        nc.vector.tensor_copy(out=sh[:, :rows], in_=s1f[:, :rows])
                        sl = sp.tile([P, 512], bf, name=f"s1l_{l}_{g0}_{mo}", tag=f"s1l{mo}")
                        nc.vector.tensor_tensor(
                            out=sl[:, :rows], in0=s1f[:, :rows], in1=sh[:, :rows], op=SUB)
                        s1h.append(sh)
                        s1l.append(sl)
                    combos2 = [("w2h", s1h), ("w2h", s1l), ("w2l", s1h)][:NSPLIT]
                    for mo in range(NKT2):
                        p2 = mlppool.tile([P, 512], f32,
                                          name=f"p2_{l}_{g0}_{mo}", tag="mlp")
                        tot = len(combos2) * NKT2
                        step = 0
                        for (wn, st) in combos2:
                            for kt in range(NKT2):
                                nc.tensor.matmul(
                                    out=p2[:, :rows],
                                    lhsT=wt[(wn, l)][:, kt * DH + mo * P: kt * DH + (mo + 1) * P],
                                    rhs=st[kt][:, :rows],
                                    start=(step == 0), stop=(step == tot - 1))
                                step += 1
                        nc.scalar.activation(
                            out=zT[mo][:, r0:r0 + rows], in_=p2[:, :rows], func=RELU,
                            bias=wt[("b2", l)][:, mo:mo + 1])

                # --- transpose back to row-major and store
                ident = identf_t if last else identb_t
                for m in range(MT):
                    rows_m = min(P, NPC - m * P)
                    tp = tppool.tile([P, NKT2 * P], f32 if last else bf,
                                     name=f"tp_{l}_{m}", tag="tp")
                    for mo in range(NKT2):
                        nc.tensor.transpose(
                            out=tp[:, mo * P:(mo + 1) * P],
                            in_=zT[mo][:, m * P:(m + 1) * P],
                            identity=ident[:])
                    zr = zrp.tile([P, NKT2 * P], f32 if last else bf,
                                  name=f"zr_{l}_{m}", tag="zr")
                    nc.vector.tensor_copy(out=zr[:], in_=tp[:])
                    dst = zout if last else zloc[l]
                    nc.sync.dma_start(
                        out=dst[m * P: m * P + rows_m, :],
                        in_=zr[:rows_m, :])

                if not last:
                    nc.gpsimd.collective_compute(
                        "AllGather",
                        mybir.AluOpType.bypass,
                        replica_groups=[list(range(NCORES))],
                        ins=[zloc[l][:, :].opt()],
                        outs=[zfull[l][:, :].opt()],
                    )

    # populates extended-inst ISA bytes + inserts GPSIMD library loads
    nc.compile()
    return nc


# --------------------------------------------------------------------------
# entry point
# --------------------------------------------------------------------------

def _make_in_maps(inputs, cfg, idx_t, slot_t):
    N, DIN, DH, L = cfg["N"], cfg["DIN"], cfg["DH"], cfg["L"]
    x_bf = np.ascontiguousarray(np.asarray(inputs["x"], dtype=np.float32)).astype(BF16)
    iota = np.broadcast_to(
        np.tile(np.arange(P, dtype=np.float32), 4), (P, 512)).astype(BF16)
    identb = np.eye(P, dtype=np.float32).astype(BF16)
    identf = np.eye(P, dtype=np.float32)

    shared = {"x_bf": x_bf, "iota": np.ascontiguousarray(iota),
              "identb": identb, "identf": identf}
    for l in range(L):
        w1 = np.asarray(inputs[f"w1_{l}"], dtype=np.float32)
        w2 = np.asarray(inputs[f"w2_{l}"], dtype=np.float32)
        w1h = w1.astype(BF16)
        w2h = w2.astype(BF16)
        shared[f"w1h_{l}"] = w1h
        shared[f"w1l_{l}"] = (w1 - w1h.astype(np.float32)).astype(BF16)
        shared[f"w2h_{l}"] = w2h
        shared[f"w2l_{l}"] = (w2 - w2h.astype(np.float32)).astype(BF16)
        shared[f"b1_{l}"] = np.asarray(
            inputs[f"b1_{l}"], dtype=np.float32).reshape(DH, 1)
        shared[f"b2_{l}"] = np.asarray(
            inputs[f"b2_{l}"], dtype=np.float32).reshape(DH, 1)

    in_maps = []
    for c in range(NCORES):
        m = dict(shared)
        m["srcidx"] = np.ascontiguousarray(idx_t[c])
        m["dstslot"] = np.ascontiguousarray(slot_t[c])
        in_maps.append(m)
    return in_maps


def get_program(inputs):
    """Build (or fetch cached) the bass program + per-core input maps."""
    cfg = _config(inputs)
    idx_t, slot_t, CPM = _prep_edges(
        inputs["edge_index"], cfg["N"], cfg["NPC"], cfg["MT"])
    key = (cfg["N"], cfg["DIN"], cfg["DH"], cfg["L"], CPM, NSPLIT)
    if key not in _BUILD_CACHE:
        _BUILD_CACHE[key] = _build(
            cfg["N"], cfg["DIN"], cfg["DH"], cfg["L"],
            cfg["NPC"], cfg["MT"], CPM)
    nc = _BUILD_CACHE[key]
    in_maps = _make_in_maps(inputs, cfg, idx_t, slot_t)
    return nc, in_maps, cfg


def kernel(**inputs):
    nc, in_maps, cfg = get_program(inputs)

    if os.environ.get("KERNEL_USE_SIM"):
        from concourse.bass_interp import MultiCoreSim
        sim = MultiCoreSim(nc, num_cores=NCORES)
        cores = list(sim.cores.values())
        for cid, cs in enumerate(cores):
            for name, val in in_maps[cid].items():
                cs.tensor(name)[:] = val
        sim.simulate(check_with_hw=False)
        parts = [np.asarray(cs.tensor("zout")) for cs in cores]
    else:
        from concourse import bass_utils
        res = bass_utils.run_bass_kernel_spmd(
            nc, in_maps, core_ids=list(range(NCORES)),
            trace=bool(os.environ.get("KERNEL_TRACE")),
        )
        kernel.last_results = res
        parts = [res.results[c]["zout"] for c in range(NCORES)]

    out = np.concatenate(parts, axis=0).astype(np.float32)
    return out


# revision 14
# speedup vs baseline: 2.2938x; 2.2938x over previous
"""Trainium2 Bass kernel for a 3-layer GIN encoder (gnn_message_passing).

Reference computation (per layer l):
    agg_i = sum_{j -> i} z_j          (scatter-add over edges)
    h     = z + agg                   (GIN eps=0, folded in as self-edges)
    z     = relu(relu(h @ w1 + b1) @ w2 + b2)

Distribution strategy (8 NeuronCores, SPMD single program):
  * Nodes are block-sharded: core c owns rows [c*NPC, (c+1)*NPC).
  * Edges are partitioned by destination core; the aggregation is local.
  * Each layer's full activation table z (bf16, row-major) lives in DRAM on
    every core (AllGather at layer boundaries = the halo exchange in the
    extreme case of a dense random graph).
  * Aggregation runs on TensorE as a dense matmul with the local adjacency
    count matrix:  h.T = z.T @ Aloc.T  where Aloc[dst_slot, src] counts
    edges (incl. one self-edge per node).  Aloc.T (bf16, exact small ints)
    is precomputed on the host and streamed from HBM in K-chunks; z sits in
    SBUF as 128-row chunks that serve as the stationary matmul operand.
    This replaces a per-edge SWDGE gather, whose Q7 descriptor generation
    (~9 ns/index) was measured as the bottleneck.
  * h.T (features on partitions) feeds the MLP directly.  The MLP runs on
    groups of 4 M-tiles (512 rows in the free dim) with hi/lo-split bf16
    matmuls (error ~= fp32) accumulated in fp32 PSUM.
  * Output rows are transposed back via TensorE and DMA'd out; layers 0..L-2
    are AllGathered into the next layer's activation table.
"""

import os
import sys

sys.path.insert(0, "/opt/trn_rl_repo")

import numpy as np
import ml_dtypes

BF16 = ml_dtypes.bfloat16
P = 128
NCORES = 8

# number of hi/lo product terms in the MLP matmuls:
# 3 = (w_hi*h_hi + w_hi*h_lo + w_lo*h_hi) ~ fp32 accuracy
# 1 = plain bf16
NSPLIT = 3

_BUILD_CACHE: dict = {}


# --------------------------------------------------------------------------
# host-side preprocessing
# --------------------------------------------------------------------------

def _config(inputs):
    x = inputs["x"]
    N, DIN = int(x.shape[0]), int(x.shape[1])
    L = 0
    while f"w1_{L}" in inputs:
        L += 1
    DH = int(inputs["w1_0"].shape[1])
    assert N % NCORES == 0
    NPC = N // NCORES
    MT = (NPC + P - 1) // P
    KC = (N + P - 1) // P
    assert DIN % P == 0 and DH % P == 0
    return dict(N=N, DIN=DIN, DH=DH, L=L, NPC=NPC, MT=MT, KC=KC)


def _prep_at(edge_index, N, NPC, MT, KC):
    """Dense transposed local adjacency per core.

    Returns at[NCORES] each [KC, 128, MT*128] bf16 with
    at[c][k, p, s] = #edges (src = k*128+p) -> (dst = c*NPC + s), plus the
    identity (self-edge).  Src rows beyond N and dst slots beyond NPC are 0.
    """
    src = np.asarray(edge_index[0], dtype=np.int64)
    dst = np.asarray(edge_index[1], dtype=np.int64)
    self_ix = np.arange(N, dtype=np.int64)
    allsrc = np.concatenate([src, self_ix])
    alldst = np.concatenate([dst, self_ix])

    core = alldst // NPC
    gslot = core * (MT * P) + (alldst - core * NPC)

    at = np.zeros((KC * P, NCORES * MT * P), np.float32)
    np.add.at(at, (allsrc, gslot), 1.0)
    at_bf = at.astype(BF16)
    out = []
    for c in range(NCORES):
        sl = at_bf[:, c * MT * P:(c + 1) * MT * P].reshape(KC, P, MT * P)
        out.append(np.ascontiguousarray(sl))
    return out


# --------------------------------------------------------------------------
# bass program
# --------------------------------------------------------------------------

def _build(N, DIN, DH, L, NPC, MT, KC):
    from concourse import bacc, mybir, tile

    f32 = mybir.dt.float32
    bf = mybir.dt.bfloat16
    SUB = mybir.AluOpType.subtract
    RELU = mybir.ActivationFunctionType.Relu

    NKT2 = DH // P  # K/M tiles of the hidden dim (2)
    GROUPS = [(g0, min(4, MT - g0)) for g0 in range(0, MT, 4)]
    # slot groups for the aggregation matmul free dim (<=512 per PSUM bank)
    NG = [(n0, min(512, MT * P - n0)) for n0 in range(0, MT * P, 512)]
    NFULL = (N // P) * P
    NREM = N - NFULL

    nc = bacc.Bacc(num_devices=NCORES)

    xin = nc.dram_tensor("x_bf", [N, DIN], bf, kind="ExternalInput")
    atin = nc.dram_tensor("at", [KC, P, MT * P], bf, kind="ExternalInput")
    identbin = nc.dram_tensor("identb", [P, P], bf, kind="ExternalInput")
    identfin = nc.dram_tensor("identf", [P, P], f32, kind="ExternalInput")
    win = {}
    for l in range(L):
        din = DIN if l == 0 else DH
        for nm, shp in [
            ("w1h", [din, DH]), ("w1l", [din, DH]),
            ("w2h", [DH, DH]), ("w2l", [DH, DH]),
        ]:
            win[(nm, l)] = nc.dram_tensor(f"{nm}_{l}", shp, bf, kind="ExternalInput")
        for nm in ("b1", "b2"):
            win[(nm, l)] = nc.dram_tensor(f"{nm}_{l}", [DH, 1], f32, kind="ExternalInput")
    zout = nc.dram_tensor("zout", [NPC, DH], f32, kind="ExternalOutput")

    with tile.TileContext(nc) as tc:
        with tc.tile_pool(name="const", bufs=1) as cp, \
             tc.tile_pool(name="atpool", bufs=4) as atp, \
             tc.tile_pool(name="zsbpool", bufs=1) as zsp, \
             tc.tile_pool(name="hpool", bufs=1) as hp, \
             tc.tile_pool(name="spool", bufs=2) as sp, \
             tc.tile_pool(name="zpool", bufs=1) as zp, \
             tc.tile_pool(name="zrpool", bufs=3) as zrp, \
             tc.tile_pool(name="hpsum", bufs=1, space="PSUM") as hpsum, \
             tc.tile_pool(name="mlppsum", bufs=2, space="PSUM") as mlppool, \
             tc.tile_pool(name="drampool", bufs=1, space="DRAM") as dp:

            # ---------------- resident constants ----------------
            identb_t = cp.tile([P, P], bf, name="identb_t")
            nc.sync.dma_start(out=identb_t[:], in_=identbin[:, :])
            identf_t = cp.tile([P, P], f32, name="identf_t")
            nc.sync.dma_start(out=identf_t[:], in_=identfin[:, :])

            wt = {}
            for l in range(L):
                din = DIN if l == 0 else DH
                nkt = din // P
                for nm in ("w1h", "w1l"):
                    t = cp.tile([P, nkt * DH], bf, name=f"{nm}{l}_t")
                    for kt in range(nkt):
                        nc.sync.dma_start(
                            out=t[:, kt * DH:(kt + 1) * DH],
                            in_=win[(nm, l)][kt * P:(kt + 1) * P, :])
                    wt[(nm, l)] = t
                for nm in ("w2h", "w2l"):
                    t = cp.tile([P, NKT2 * DH], bf, name=f"{nm}{l}_t")
                    for kt in range(NKT2):
                        nc.sync.dma_start(
                            out=t[:, kt * DH:(kt + 1) * DH],
                            in_=win[(nm, l)][kt * P:(kt + 1) * P, :])
                    wt[(nm, l)] = t
                for nm in ("b1", "b2"):
                    t = cp.tile([P, NKT2], f32, name=f"{nm}{l}_t")
                    for mo in range(NKT2):
                        nc.sync.dma_start(
                            out=t[:, mo:mo + 1],
                            in_=win[(nm, l)][mo * P:(mo + 1) * P, :])
                    wt[(nm, l)] = t

            # layer-boundary activation tables
            zloc = [dp.tile([NPC, DH], bf, name=f"zloc{l}") for l in range(L - 1)]
            zfull = [dp.tile([N, DH], bf, name=f"zfull{l}", addr_space="Shared")
                     for l in range(L - 1)]

            # ---------------- layers ----------------
            for l in range(L):
                din = DIN if l == 0 else DH
                nkt = din // P
                last = (l == L - 1)
                table = xin[:, :] if l == 0 else zfull[l - 1][:, :]

                # activation table -> SBUF, chunked [128, KC*din]:
                # zsb[p, k*din+f] = z[k*128+p, f]
                zsb = zsp.tile([P, KC * din], bf, name=f"zsb_{l}", tag="zsb")
                nc.sync.dma_start(
                    out=zsb[:, :(N // P) * din].rearrange("p (k f) -> p k f", f=din),
                    in_=table[:NFULL, :].rearrange("(k p) f -> p k f", p=P))
                if NREM:
                    nc.vector.memset(zsb[:, (N // P) * din:], 0.0)
                    nc.sync.dma_start(
                        out=zsb[:NREM, (N // P) * din:],
                        in_=table[NFULL:, :])

                # --- aggregation: h.T = z.T @ Aloc.T  (PSUM-accumulated)
                hps = [hpsum.tile([P, len(NG) * 512], f32,
                                  name=f"hps{mf}_{l}", tag=f"hps{mf}")
                       for mf in range(nkt)]
                for k in range(KC):
                    at_t = atp.tile([P, MT * P], bf, name=f"at_{l}_{k}", tag="at")
                    nc.sync.dma_start(out=at_t[:], in_=atin[k, :, :])
                    for mf in range(nkt):
                        for gi, (n0, nn) in enumerate(NG):
                            nc.tensor.matmul(
                                out=hps[mf][:, gi * 512: gi * 512 + nn],
                                lhsT=zsb[:, k * din + mf * P: k * din + (mf + 1) * P],
                                rhs=at_t[:, n0:n0 + nn],
                                start=(k == 0),
                                stop=(k == KC - 1),
                            )

                # --- split h.T into hi/lo bf16
                hhi = [hp.tile([P, MT * P], bf, name=f"hhi{mf}_{l}", tag=f"hhi{mf}")
                       for mf in range(nkt)]
                hlo = [hp.tile([P, MT * P], bf, name=f"hlo{mf}_{l}", tag=f"hlo{mf}")
                       for mf in range(nkt)]
                for mf in range(nkt):
                    for gi, (n0, nn) in enumerate(NG):
                        nc.vector.tensor_copy(
                            out=hhi[mf][:, n0:n0 + nn],
                            in_=hps[mf][:, gi * 512: gi * 512 + nn])
                        nc.vector.tensor_tensor(
                            out=hlo[mf][:, n0:n0 + nn],
                            in0=hps[mf][:, gi * 512: gi * 512 + nn],
                            in1=hhi[mf][:, n0:n0 + nn],
                            op=SUB)

                # --- MLP over groups of 4 M-tiles (512-row free dim)
                zT = [zp.tile([P, MT * P], f32 if last else bf,
                              name=f"zT{mo}_{l}",
                              tag=f"zT{mo}{'f' if last else 'b'}")
                      for mo in range(NKT2)]
                for (g0, gm) in GROUPS:
                    rows = gm * P
                    r0 = g0 * P
                    combos1 = [("w1h", hhi), ("w1h", hlo), ("w1l", hhi)][:NSPLIT]
                    s1h, s1l = [], []
                    for mo in range(NKT2):
                        p1 = mlppool.tile([P, 512], f32,
                                          name=f"p1_{l}_{g0}_{mo}", tag="mlp")
                        tot = len(combos1) * nkt
                        step = 0
                        for (wn, ht) in combos1:
                            for kt in range(nkt):
                                nc.tensor.matmul(
                                    out=p1[:, :rows],
                                    lhsT=wt[(wn, l)][:, kt * DH + mo * P: kt * DH + (mo + 1) * P],
                                    rhs=ht[kt][:, r0:r0 + rows],
                                    start=(step == 0), stop=(step == tot - 1))
                                step += 1
                        s1f = sp.tile([P, 512], f32, name=f"s1f_{l}_{g0}_{mo}", tag="s1f")
                        nc.scalar.activation(
                            out=s1f[:, :rows], in_=p1[:, :rows], func=RELU,
                            bias=wt[("b1", l)][:, mo:mo + 1])
                        sh = sp.tile([P, 512], bf, name=f"s1h_{l}_{g0}_{mo}", tag=f"s1h{mo}")
                        nc.vector.tensor_copy(out=sh[:, :rows], in_=s1f[:, :rows])
                        sl = sp.tile([P, 512], bf, name=f"s1l_{l}_{g0}_{mo}", tag=f"s1l{mo}")
                        nc.vector.tensor_tensor(
                            out=sl[:, :rows], in0=s1f[:, :rows], in1=sh[:, :rows], op=SUB)
                        s1h.append(sh)
                        s1l.append(sl)
                    combos2 = [("w2h", s1h), ("w2h", s1l), ("w2l", s1h)][:NSPLIT]
                    for mo in range(NKT2):
                        p2 = mlppool.tile([P, 512], f32,
                                          name=f"p2_{l}_{g0}_{mo}", tag="mlp")
                        tot = len(combos2) * NKT2
                        step = 0
                        for (wn, st) in combos2:
                            for kt in range(NKT2):
                                nc.tensor.matmul(
                                    out=p2[:, :rows],
                                    lhsT=wt[(wn, l)][:, kt * DH + mo * P: kt * DH + (mo + 1) * P],
                                    rhs=st[kt][:, :rows],
                                    start=(step == 0), stop=(step == tot - 1))
                                step += 1
                        nc.scalar.activation(
                            out=zT[mo][:, r0:r0 + rows], in_=p2[:, :rows], func=RELU,
                            bias=wt[("b2", l)][:, mo:mo + 1])

                # --- transpose back to row-major and store
                ident = identf_t if last else identb_t
                for m in range(MT):
                    rows_m = min(P, NPC - m * P)
                    tp = mlppool.tile([P, NKT2 * P], f32 if last else bf,
                                      name=f"tp_{l}_{m}", tag="mlp")
                    for mo in range(NKT2):
                        nc.tensor.transpose(
                            out=tp[:, mo * P:(mo + 1) * P],
                            in_=zT[mo][:, m * P:(m + 1) * P],
                            identity=ident[:])
                    zr = zrp.tile([P, NKT2 * P], f32 if last else bf,
                                  name=f"zr_{l}_{m}", tag="zr")
                    nc.vector.tensor_copy(out=zr[:], in_=tp[:])
                    dst = zout if last else zloc[l]
                    nc.sync.dma_start(
                        out=dst[m * P: m * P + rows_m, :],
                        in_=zr[:rows_m, :])

                if not last:
                    nc.gpsimd.collective_compute(
                        "AllGather",
                        mybir.AluOpType.bypass,
                        replica_groups=[list(range(NCORES))],
                        ins=[zloc[l][:, :].opt()],
                        outs=[zfull[l][:, :].opt()],
                    )

    # populates extended-inst ISA bytes + inserts GPSIMD library loads
    nc.compile()
    return nc


# --------------------------------------------------------------------------
# entry point
# --------------------------------------------------------------------------

def _make_in_maps(inputs, cfg, at):
    DH, L = cfg["DH"], cfg["L"]
    x_bf = np.ascontiguousarray(np.asarray(inputs["x"], dtype=np.float32)).astype(BF16)
    identb = np.eye(P, dtype=np.float32).astype(BF16)
    identf = np.eye(P, dtype=np.float32)

    shared = {"x_bf": x_bf, "identb": identb, "identf": identf}
    for l in range(L):
        w1 = np.asarray(inputs[f"w1_{l}"], dtype=np.float32)
        w2 = np.asarray(inputs[f"w2_{l}"], dtype=np.float32)
        w1h = w1.astype(BF16)
        w2h = w2.astype(BF16)
        shared[f"w1h_{l}"] = w1h
        shared[f"w1l_{l}"] = (w1 - w1h.astype(np.float32)).astype(BF16)
        shared[f"w2h_{l}"] = w2h
        shared[f"w2l_{l}"] = (w2 - w2h.astype(np.float32)).astype(BF16)
        shared[f"b1_{l}"] = np.asarray(
            inputs[f"b1_{l}"], dtype=np.float32).reshape(DH, 1)
        shared[f"b2_{l}"] = np.asarray(
            inputs[f"b2_{l}"], dtype=np.float32).reshape(DH, 1)

    in_maps = []
    for c in range(NCORES):
        m = dict(shared)
        m["at"] = at[c]
        in_maps.append(m)
    return in_maps


def get_program(inputs):
    """Build (or fetch cached) the bass program + per-core input maps."""
    cfg = _config(inputs)
    at = _prep_at(inputs["edge_index"], cfg["N"], cfg["NPC"], cfg["MT"], cfg["KC"])
    key = (cfg["N"], cfg["DIN"], cfg["DH"], cfg["L"], NSPLIT)
    if key not in _BUILD_CACHE:
        _BUILD_CACHE[key] = _build(
            cfg["N"], cfg["DIN"], cfg["DH"], cfg["L"],
            cfg["NPC"], cfg["MT"], cfg["KC"])
    nc = _BUILD_CACHE[key]
    in_maps = _make_in_maps(inputs, cfg, at)
    return nc, in_maps, cfg


def kernel(**inputs):
    nc, in_maps, cfg = get_program(inputs)

    if os.environ.get("KERNEL_USE_SIM"):
        from concourse.bass_interp import MultiCoreSim
        sim = MultiCoreSim(nc, num_cores=NCORES)
        cores = list(sim.cores.values())
        for cid, cs in enumerate(cores):
            for name, val in in_maps[cid].items():
                cs.tensor(name)[:] = val
        sim.simulate(check_with_hw=False)
        parts = [np.asarray(cs.tensor("zout")) for cs in cores]
    else:
        from concourse import bass_utils
        res = bass_utils.run_bass_kernel_spmd(
            nc, in_maps, core_ids=list(range(NCORES)),
            trace=bool(os.environ.get("KERNEL_TRACE")),
        )
        kernel.last_results = res
        parts = [res.results[c]["zout"] for c in range(NCORES)]

    out = np.concatenate(parts, axis=0).astype(np.float32)
    return out


# revision 18
# speedup vs baseline: 2.7641x; 1.2050x over previous
"""Trainium2 Bass kernel for a 3-layer GIN encoder (gnn_message_passing).

Reference computation (per layer l):
    agg_i = sum_{j -> i} z_j          (scatter-add over edges)
    h     = z + agg                   (GIN eps=0, folded in as self-edges)
    z     = relu(relu(h @ w1 + b1) @ w2 + b2)

Distribution strategy (8 NeuronCores, SPMD single program):
  * Nodes are block-sharded: core c owns rows [c*NPC, (c+1)*NPC).
  * Edges are partitioned by destination core; the aggregation is local.
  * Each layer's full activation table z (bf16, row-major) lives in DRAM on
    every core (AllGather at layer boundaries = the halo exchange in the
    extreme case of a dense random graph).
  * Aggregation runs on TensorE as a dense matmul with the local adjacency
    count matrix:  h.T = z.T @ Aloc.T  where Aloc[dst_slot, src] counts
    edges (incl. one self-edge per node).  Aloc.T (bf16, exact small ints)
    is precomputed on the host and streamed from HBM in K-chunks; z sits in
    SBUF as 128-row chunks that serve as the stationary matmul operand.
    This replaces a per-edge SWDGE gather, whose Q7 descriptor generation
    (~9 ns/index) was measured as the bottleneck.
  * h.T (features on partitions) feeds the MLP directly.  The MLP runs on
    groups of 4 M-tiles (512 rows in the free dim) with hi/lo-split bf16
    matmuls (error ~= fp32) accumulated in fp32 PSUM.
  * Output rows are transposed back via TensorE and DMA'd out; layers 0..L-2
    are AllGathered into the next layer's activation table.
"""

import os
import sys

sys.path.insert(0, "/opt/trn_rl_repo")

import numpy as np
import ml_dtypes

BF16 = ml_dtypes.bfloat16
P = 128
NCORES = 8

# number of hi/lo product terms in the MLP matmuls:
# 3 = (w_hi*h_hi + w_hi*h_lo + w_lo*h_hi) ~ fp32 accuracy
# 1 = plain bf16
NSPLIT = 3

# adjacency K-chunks fetched per DMA (batching amortizes HWDGE issue cost)
ABATCH = 2

_BUILD_CACHE: dict = {}


# --------------------------------------------------------------------------
# host-side preprocessing
# --------------------------------------------------------------------------

def _config(inputs):
    x = inputs["x"]
    N, DIN = int(x.shape[0]), int(x.shape[1])
    L = 0
    while f"w1_{L}" in inputs:
        L += 1
    DH = int(inputs["w1_0"].shape[1])
    assert N % NCORES == 0
    NPC = N // NCORES
    MT = (NPC + P - 1) // P
    KC = (N + P - 1) // P
    assert DIN % P == 0 and DH % P == 0
    return dict(N=N, DIN=DIN, DH=DH, L=L, NPC=NPC, MT=MT, KC=KC)


def _prep_at(edge_index, N, NPC, MT, KC):
    """Dense transposed local adjacency per core.

    Returns at[NCORES] each [KC, 128, MT*128] bf16 with
    at[c][k, p, s] = #edges (src = k*128+p) -> (dst = c*NPC + s), plus the
    identity (self-edge).  Src rows beyond N and dst slots beyond NPC are 0.
    """
    src = np.asarray(edge_index[0], dtype=np.int64)
    dst = np.asarray(edge_index[1], dtype=np.int64)
    self_ix = np.arange(N, dtype=np.int64)
    allsrc = np.concatenate([src, self_ix])
    alldst = np.concatenate([dst, self_ix])

    core = alldst // NPC
    gslot = core * (MT * P) + (alldst - core * NPC)

    at = np.zeros((KC * P, NCORES * MT * P), np.float32)
    np.add.at(at, (allsrc, gslot), 1.0)
    at_bf = at.astype(BF16)
    out = []
    for c in range(NCORES):
        sl = at_bf[:, c * MT * P:(c + 1) * MT * P].reshape(KC, P, MT * P)
        out.append(np.ascontiguousarray(sl))
    return out


# --------------------------------------------------------------------------
# bass program
# --------------------------------------------------------------------------

def _build(N, DIN, DH, L, NPC, MT, KC):
    from concourse import bacc, mybir, tile

    f32 = mybir.dt.float32
    bf = mybir.dt.bfloat16
    SUB = mybir.AluOpType.subtract
    RELU = mybir.ActivationFunctionType.Relu

    NKT2 = DH // P  # K/M tiles of the hidden dim (2)
    GROUPS = [(g0, min(4, MT - g0)) for g0 in range(0, MT, 4)]
    # slot groups for the aggregation matmul free dim (<=512 per PSUM bank)
    NG = [(n0, min(512, MT * P - n0)) for n0 in range(0, MT * P, 512)]
    NFULL = (N // P) * P
    NREM = N - NFULL

    nc = bacc.Bacc(num_devices=NCORES)

    xin = nc.dram_tensor("x_bf", [N, DIN], bf, kind="ExternalInput")
    atin = nc.dram_tensor("at", [KC, P, MT * P], bf, kind="ExternalInput")
    identbin = nc.dram_tensor("identb", [P, P], bf, kind="ExternalInput")
    identfin = nc.dram_tensor("identf", [P, P], f32, kind="ExternalInput")
    win = {}
    for l in range(L):
        din = DIN if l == 0 else DH
        for nm, shp in [
            ("w1h", [din, DH]), ("w1l", [din, DH]),
            ("w2h", [DH, DH]), ("w2l", [DH, DH]),
        ]:
            win[(nm, l)] = nc.dram_tensor(f"{nm}_{l}", shp, bf, kind="ExternalInput")
        for nm in ("b1", "b2"):
            win[(nm, l)] = nc.dram_tensor(f"{nm}_{l}", [DH, 1], f32, kind="ExternalInput")
    zout = nc.dram_tensor("zout", [NPC, DH], f32, kind="ExternalOutput")

    with tile.TileContext(nc) as tc:
        with tc.tile_pool(name="const", bufs=1) as cp, \
             tc.tile_pool(name="atpool", bufs=6) as atp, \
             tc.tile_pool(name="zsbpool", bufs=1) as zsp, \
             tc.tile_pool(name="hpool", bufs=1) as hp, \
             tc.tile_pool(name="spool", bufs=2) as sp, \
             tc.tile_pool(name="zpool", bufs=1) as zp, \
             tc.tile_pool(name="zrpool", bufs=3) as zrp, \
             tc.tile_pool(name="hpsum", bufs=1, space="PSUM") as hpsum, \
             tc.tile_pool(name="mlppsum", bufs=2, space="PSUM") as mlppool, \
             tc.tile_pool(name="drampool", bufs=1, space="DRAM") as dp:

            # ---------------- resident constants ----------------
            identb_t = cp.tile([P, P], bf, name="identb_t")
            nc.sync.dma_start(out=identb_t[:], in_=identbin[:, :])
            identf_t = cp.tile([P, P], f32, name="identf_t")
            nc.sync.dma_start(out=identf_t[:], in_=identfin[:, :])

            wt = {}
            for l in range(L):
                din = DIN if l == 0 else DH
                nkt = din // P
                for nm in ("w1h", "w1l"):
                    t = cp.tile([P, nkt * DH], bf, name=f"{nm}{l}_t")
                    for kt in range(nkt):
                        nc.sync.dma_start(
                            out=t[:, kt * DH:(kt + 1) * DH],
                            in_=win[(nm, l)][kt * P:(kt + 1) * P, :])
                    wt[(nm, l)] = t
                for nm in ("w2h", "w2l"):
                    t = cp.tile([P, NKT2 * DH], bf, name=f"{nm}{l}_t")
                    for kt in range(NKT2):
                        nc.sync.dma_start(
                            out=t[:, kt * DH:(kt + 1) * DH],
                            in_=win[(nm, l)][kt * P:(kt + 1) * P, :])
                    wt[(nm, l)] = t
                for nm in ("b1", "b2"):
                    t = cp.tile([P, NKT2], f32, name=f"{nm}{l}_t")
                    for mo in range(NKT2):
                        nc.sync.dma_start(
                            out=t[:, mo:mo + 1],
                            in_=win[(nm, l)][mo * P:(mo + 1) * P, :])
                    wt[(nm, l)] = t

            # layer-boundary activation tables
            zloc = [dp.tile([NPC, DH], bf, name=f"zloc{l}") for l in range(L - 1)]
            zfull = [dp.tile([N, DH], bf, name=f"zfull{l}", addr_space="Shared")
                     for l in range(L - 1)]

            # ---------------- layers ----------------
            for l in range(L):
                din = DIN if l == 0 else DH
                nkt = din // P
                last = (l == L - 1)
                table = xin[:, :] if l == 0 else zfull[l - 1][:, :]

                # activation table -> SBUF, chunked [128, KC*din]:
                # zsb[p, k*din+f] = z[k*128+p, f]; split into pieces so the
                # K-loop matmuls can start before the whole table landed
                zsb = zsp.tile([P, KC * din], bf, name=f"zsb_{l}", tag="zsb")
                KFULL = N // P
                ZPIECE = 16
                for z0 in range(0, KFULL, ZPIECE):
                    z1 = min(KFULL, z0 + ZPIECE)
                    eng = nc.sync if (z0 // ZPIECE) % 2 == 0 else nc.scalar
                    eng.dma_start(
                        out=zsb[:, z0 * din: z1 * din]
                            .rearrange("p (k f) -> p k f", f=din),
                        in_=table[z0 * P: z1 * P, :]
                            .rearrange("(k p) f -> p k f", p=P))
                if NREM:
                    nc.vector.memset(zsb[:, KFULL * din:], 0.0)
                    nc.sync.dma_start(
                        out=zsb[:NREM, KFULL * din:],
                        in_=table[NFULL:, :])

                # --- aggregation: h.T = z.T @ Aloc.T  (PSUM-accumulated)
                hps = [hpsum.tile([P, len(NG) * 512], f32,
                                  name=f"hps{mf}_{l}", tag=f"hps{mf}")
                       for mf in range(nkt)]
                for kb in range(0, KC, ABATCH):
                    ke = min(KC, kb + ABATCH)
                    at_t = atp.tile([P, ABATCH * MT * P], bf,
                                    name=f"at_{l}_{kb}", tag="at")
                    eng = nc.sync if (kb // ABATCH) % 2 == 0 else nc.scalar
                    eng.dma_start(
                        out=at_t[:, :(ke - kb) * MT * P]
                            .rearrange("p (k n) -> p k n", n=MT * P),
                        in_=atin[kb:ke, :, :].rearrange("k p n -> p k n"))
                    for k in range(kb, ke):
                        for mf in range(nkt):
                            for gi, (n0, nn) in enumerate(NG):
                                nc.tensor.matmul(
                                    out=hps[mf][:, gi * 512: gi * 512 + nn],
                                    lhsT=zsb[:, k * din + mf * P: k * din + (mf + 1) * P],
                                    rhs=at_t[:, (k - kb) * MT * P + n0:
                                             (k - kb) * MT * P + n0 + nn],
                                    start=(k == 0),
                                    stop=(k == KC - 1),
                                )

                # --- split h.T into hi/lo bf16
                hhi = [hp.tile([P, MT * P], bf, name=f"hhi{mf}_{l}", tag=f"hhi{mf}")
                       for mf in range(nkt)]
                hlo = [hp.tile([P, MT * P], bf, name=f"hlo{mf}_{l}", tag=f"hlo{mf}")
                       for mf in range(nkt)]
                for mf in range(nkt):
                    for gi, (n0, nn) in enumerate(NG):
                        nc.vector.tensor_copy(
                            out=hhi[mf][:, n0:n0 + nn],
                            in_=hps[mf][:, gi * 512: gi * 512 + nn])
                        nc.vector.tensor_tensor(
                            out=hlo[mf][:, n0:n0 + nn],
                            in0=hps[mf][:, gi * 512: gi * 512 + nn],
                            in1=hhi[mf][:, n0:n0 + nn],
                            op=SUB)

                # --- MLP over groups of 4 M-tiles (512-row free dim)
                zT = [zp.tile([P, MT * P], f32 if last else bf,
                              name=f"zT{mo}_{l}",
                              tag=f"zT{mo}{'f' if last else 'b'}")
                      for mo in range(NKT2)]
                for (g0, gm) in GROUPS:
                    rows = gm * P
                    r0 = g0 * P
                    combos1 = [("w1h", hhi), ("w1h", hlo), ("w1l", hhi)][:NSPLIT]
                    s1h, s1l = [], []
                    for mo in range(NKT2):
                        p1 = mlppool.tile([P, 512], f32,
                                          name=f"p1_{l}_{g0}_{mo}", tag="mlp")
                        tot = len(combos1) * nkt
                        step = 0
                        for (wn, ht) in combos1:
                            for kt in range(nkt):
                                nc.tensor.matmul(
                                    out=p1[:, :rows],
                                    lhsT=wt[(wn, l)][:, kt * DH + mo * P: kt * DH + (mo + 1) * P],
                                    rhs=ht[kt][:, r0:r0 + rows],
                                    start=(step == 0), stop=(step == tot - 1))
                                step += 1
                        s1f = sp.tile([P, 512], f32, name=f"s1f_{l}_{g0}_{mo}", tag="s1f")
                        nc.scalar.activation(
                            out=s1f[:, :rows], in_=p1[:, :rows], func=RELU,
                            bias=wt[("b1", l)][:, mo:mo + 1])
                        sh = sp.tile([P, 512], bf, name=f"s1h_{l}_{g0}_{mo}", tag=f"s1h{mo}")
                        nc.vector.tensor_copy(out=sh[:, :rows], in_=s1f[:, :rows])
                        sl = sp.tile([P, 512], bf, name=f"s1l_{l}_{g0}_{mo}", tag=f"s1l{mo}")
                        nc.vector.tensor_tensor(
                            out=sl[:, :rows], in0=s1f[:, :rows], in1=sh[:, :rows], op=SUB)
                        s1h.append(sh)
                        s1l.append(sl)
                    combos2 = [("w2h", s1h), ("w2h", s1l), ("w2l", s1h)][:NSPLIT]
                    for mo in range(NKT2):
                        p2 = mlppool.tile([P, 512], f32,
                                          name=f"p2_{l}_{g0}_{mo}", tag="mlp")
                        tot = len(combos2) * NKT2
                        step = 0
                        for (wn, st) in combos2:
                            for kt in range(NKT2):
                                nc.tensor.matmul(
                                    out=p2[:, :rows],
                                    lhsT=wt[(wn, l)][:, kt * DH + mo * P: kt * DH + (mo + 1) * P],
                                    rhs=st[kt][:, :rows],
                                    start=(step == 0), stop=(step == tot - 1))
                                step += 1
                        nc.scalar.activation(
                            out=zT[mo][:, r0:r0 + rows], in_=p2[:, :rows], func=RELU,
                            bias=wt[("b2", l)][:, mo:mo + 1])

                # --- transpose back to row-major and store
                ident = identf_t if last else identb_t
                for m in range(MT):
                    rows_m = min(P, NPC - m * P)
                    tp = mlppool.tile([P, NKT2 * P], f32 if last else bf,
                                      name=f"tp_{l}_{m}", tag="mlp")
                    for mo in range(NKT2):
                        nc.tensor.transpose(
                            out=tp[:, mo * P:(mo + 1) * P],
                            in_=zT[mo][:, m * P:(m + 1) * P],
                            identity=ident[:])
                    zr = zrp.tile([P, NKT2 * P], f32 if last else bf,
                                  name=f"zr_{l}_{m}", tag="zr")
                    nc.vector.tensor_copy(out=zr[:], in_=tp[:])
                    dst = zout if last else zloc[l]
                    nc.sync.dma_start(
                        out=dst[m * P: m * P + rows_m, :],
                        in_=zr[:rows_m, :])

                if not last:
                    nc.gpsimd.collective_compute(
                        "AllGather",
                        mybir.AluOpType.bypass,
                        replica_groups=[list(range(NCORES))],
                        ins=[zloc[l][:, :].opt()],
                        outs=[zfull[l][:, :].opt()],
                    )

    # populates extended-inst ISA bytes + inserts GPSIMD library loads
    nc.compile()
    return nc


# --------------------------------------------------------------------------
# entry point
# --------------------------------------------------------------------------

def _make_in_maps(inputs, cfg, at):
    DH, L = cfg["DH"], cfg["L"]
    x_bf = np.ascontiguousarray(np.asarray(inputs["x"], dtype=np.float32)).astype(BF16)
    identb = np.eye(P, dtype=np.float32).astype(BF16)
    identf = np.eye(P, dtype=np.float32)

    shared = {"x_bf": x_bf, "identb": identb, "identf": identf}
    for l in range(L):
        w1 = np.asarray(inputs[f"w1_{l}"], dtype=np.float32)
        w2 = np.asarray(inputs[f"w2_{l}"], dtype=np.float32)
        w1h = w1.astype(BF16)
        w2h = w2.astype(BF16)
        shared[f"w1h_{l}"] = w1h
        shared[f"w1l_{l}"] = (w1 - w1h.astype(np.float32)).astype(BF16)
        shared[f"w2h_{l}"] = w2h
        shared[f"w2l_{l}"] = (w2 - w2h.astype(np.float32)).astype(BF16)
        shared[f"b1_{l}"] = np.asarray(
            inputs[f"b1_{l}"], dtype=np.float32).reshape(DH, 1)
        shared[f"b2_{l}"] = np.asarray(
            inputs[f"b2_{l}"], dtype=np.float32).reshape(DH, 1)

    in_maps = []
    for c in range(NCORES):
        m = dict(shared)
        m["at"] = at[c]
        in_maps.append(m)
    return in_maps


def get_program(inputs):
    """Build (or fetch cached) the bass program + per-core input maps."""
    cfg = _config(inputs)
    at = _prep_at(inputs["edge_index"], cfg["N"], cfg["NPC"], cfg["MT"], cfg["KC"])
    key = (cfg["N"], cfg["DIN"], cfg["DH"], cfg["L"], NSPLIT)
    if key not in _BUILD_CACHE:
        _BUILD_CACHE[key] = _build(
            cfg["N"], cfg["DIN"], cfg["DH"], cfg["L"],
            cfg["NPC"], cfg["MT"], cfg["KC"])
    nc = _BUILD_CACHE[key]
    in_maps = _make_in_maps(inputs, cfg, at)
    return nc, in_maps, cfg


def kernel(**inputs):
    nc, in_maps, cfg = get_program(inputs)

    if os.environ.get("KERNEL_USE_SIM"):
        from concourse.bass_interp import MultiCoreSim
        sim = MultiCoreSim(nc, num_cores=NCORES)
        cores = list(sim.cores.values())
        for cid, cs in enumerate(cores):
            for name, val in in_maps[cid].items():
                cs.tensor(name)[:] = val
        sim.simulate(check_with_hw=False)
        parts = [np.asarray(cs.tensor("zout")) for cs in cores]
    else:
        from concourse import bass_utils
        res = bass_utils.run_bass_kernel_spmd(
            nc, in_maps, core_ids=list(range(NCORES)),
            trace=bool(os.environ.get("KERNEL_TRACE")),
        )
        kernel.last_results = res
        parts = [res.results[c]["zout"] for c in range(NCORES)]

    out = np.concatenate(parts, axis=0).astype(np.float32)
    return out
